# revision 12
# baseline (speedup 1.0000x reference)
"""Trainium2 Bass kernel for per-batch spatial self-attention — fp8 DoubleRow.

Per-core computation (one batch image per NeuronCore, 8 cores):
  x:(256, 4096) bf16 -> q/k = W x + b (channels-major, fp8e4m3 + residual qr16)
                        v = x^T W^T + b (pixels-major, fp8e4m3)
  St[m,n] = sum_c K8[c,m] Q8[c,n]  fp8 DoubleRow (256-deep contraction/instr)
  Pt = exp(St/16) -> fp8           one Act instruction per 2-bank PSUM tile
  OT[o,n] = sum_m V[m,o] Pt[m,n]   fp8 DoubleRow, key pairs
  rowsum via fp8 DoubleRow ones-matmul accumulated across key pairs
  out = OT/rowsum + (dq @ G)/N     Gram correction: G = K^T V / 16 cancels the
                                   query-side fp8 quantization error.
  qr16 = fp8(16*(q - q8)) is the stored q-residual; G accumulates in the spare
  PSUM bank during tiles 0-1 from a pixels-major K projection (kt8) and v8.

Engine placement: projections run in bf16 on the PE (1 cycle/row); exp and the
correction scale on Act; q8/k8 quantization and the normalize chain on DVE;
v8/kt8/u PSUM->SBUF moves on gpsimd (idle otherwise — all input DMAs ride the
hw DGE sync queue). x and the conv weights are pre-cast to bf16 on the host.
"""

import sys

sys.path.insert(0, "/opt/trn_rl_repo")

import numpy as np
import ml_dtypes
import concourse.bacc as bacc
import concourse.bass as bass
import concourse.mybir as mybir
import concourse.tile as tile
from concourse.bass_utils import run_bass_kernel_spmd

F32 = mybir.dt.float32
FP8 = mybir.dt.float8e4
BF16 = mybir.dt.bfloat16
FP16 = mybir.dt.float16
AF = mybir.ActivationFunctionType
DR = mybir.MatmulPerfMode.DoubleRow
MULT = mybir.AluOpType.mult
ADD = mybir.AluOpType.add

B = 8
C = 256  # channels
NPIX = 4096  # 64*64
NT = NPIX // 512  # 8 query tiles of 512
NJ = NPIX // 256  # 16 key-chunk pairs (2x128) per query tile
SCALE = 1.0 / 16.0  # 1/sqrt(C)
LAG = 4  # OT stream lags the score stream by this many (nt,j) steps
EPI_DELAY = 2  # epilogue part B deferred this many steps (G-gated for nt 0/1)

_CACHE = {}


def _build():
    nc = bacc.Bacc("TRN2", num_swdge_queues=4)
    x_d = nc.declare_dram_parameter("x", [C, NPIX], BF16, isOutput=False)
    wq_d = nc.declare_dram_parameter("wq_t", [C, C], BF16, isOutput=False)
    wk_d = nc.declare_dram_parameter("wk_t", [C, C], BF16, isOutput=False)
    wv_d = nc.declare_dram_parameter("wv_t", [C, C], BF16, isOutput=False)
    bq_d = nc.declare_dram_parameter("bq", [C, 1], F32, isOutput=False)
    bv_d = nc.declare_dram_parameter("bv", [1, C], F32, isOutput=False)
    out_d = nc.declare_dram_parameter("out", [C, NPIX], F32, isOutput=True)

    with tile.TileContext(nc) as tc:
        with (
            tc.tile_pool(name="big", bufs=1) as big,
            tc.tile_pool(name="small", bufs=2) as small,
            tc.tile_pool(name="ptp", bufs=LAG + 3) as ptp,
            tc.tile_pool(name="up", bufs=3) as up,
            tc.tile_pool(name="osbp", bufs=2) as osbp,
            tc.tile_pool(name="stp", bufs=2, space="PSUM") as stp,
            tc.tile_pool(name="psO", bufs=1, space="PSUM") as psO,
            tc.tile_pool(name="psR", bufs=1, space="PSUM") as psR,
            tc.tile_pool(name="psQ", bufs=1, space="PSUM") as psQ,
        ):
            # ---- warm-up tiles first: their DVE ops have no DMA deps, so
            # the PE p-state ramp starts immediately ----
            warm_f = small.tile([128, 256], F32, name="warm_f", tag="warm_f")
            nc.vector.memset(warm_f, 1.0)
            warm_r = small.tile([128, 256], BF16, name="warm_r", tag="warm_r")
            nc.vector.tensor_copy(warm_r, warm_f)
            warm_c = small.tile([128, 1], BF16, name="warm_c", tag="warm_c")
            nc.vector.tensor_copy(warm_c, warm_f[:, 0:1])
            warm_ps = stp.tile([128, 2, 512], F32, name="warm_ps", tag="st")
            for _ in range(12):
                nc.tensor.matmul(
                    warm_ps[0:1, 0, 0:256],
                    warm_c,
                    warm_r,
                    start=True,
                    stop=True,
                    skip_group_check=True,
                )

            # ---- input DMAs on the hw DGE (sync) queue, need-ordered;
            # the gpsimd/swdge queues stay free for compute offload ----
            w_b = {}
            for nm in ("q", "k", "v"):
                w_b[nm] = [
                    big.tile([128, C], BF16, name=f"w{nm}_b{i}") for i in range(2)
                ]
            x_b = [big.tile([128, NPIX], BF16, name=f"x_b{i}") for i in range(2)]

            wd = {"q": wq_d, "k": wk_d, "v": wv_d}
            for nm in ("k", "q", "v"):
                for i in range(2):
                    nc.sync.dma_start(
                        out=w_b[nm][i], in_=wd[nm][i * 128 : (i + 1) * 128, :]
                    )
            bq_sb = [big.tile([128, 1], F32, name=f"bq_sb{i}") for i in range(2)]
            for i in range(2):
                nc.sync.dma_start(out=bq_sb[i], in_=bq_d[i * 128 : (i + 1) * 128, :])
            # bv as per-partition columns (added via the corr Act bias: the
            # V-bias passes through row-normalized attention unchanged)
            bv_col = [big.tile([128, 1], F32, name=f"bv_col{i}") for i in range(2)]
            for i in range(2):
                bv_col_ap = bass.AP(
                    tensor=bv_d.ap().tensor,
                    offset=i * 128,
                    ap=[[1, 128], [0, 1]],
                )
                nc.sync.dma_start(out=bv_col[i], in_=bv_col_ap)
            for half in range(4):
                for i in range(2):
                    lo, hi = half * 256, (half + 1) * 256
                    nc.sync.dma_start(
                        out=x_b[i][:, lo:hi], in_=x_d[i * 128 : (i + 1) * 128, lo:hi]
                    )
            for s in range(2, NT):
                lo, hi = s * 512, (s + 1) * 512
                for i in range(2):
                    nc.sync.dma_start(
                        out=x_b[i][:, lo:hi], in_=x_d[i * 128 : (i + 1) * 128, lo:hi]
                    )

            bq16_sb = [big.tile([128, 1], F32, name=f"bq16_sb{i}") for i in range(2)]

            # constants (DVE ops emitted lazily inside the step loop so the
            # first k8/q8 casts aren't queued behind them)
            ones_f2 = big.tile([128, 2, 32], F32, name="ones_f2")
            ones8 = big.tile([128, 2, 32], FP8, name="ones8")
            ones_rf = big.tile([1, 128], F32, name="ones_rf")
            ones_row = big.tile([1, 128], FP16, name="ones_row")

            def emit_ones8():
                nc.vector.memset(ones_f2, 1.0)
                nc.vector.tensor_copy(ones8, ones_f2)

            def emit_ones_row():
                nc.vector.memset(ones_rf, 1.0)
                nc.vector.tensor_copy(ones_row, ones_rf)

            # fp8 storage
            q8 = big.tile([128, 2, NPIX], FP8, name="q8")
            qr16 = big.tile([128, 2, NPIX], FP8, name="qr16")
            k8 = big.tile([128, 2, NPIX], FP8, name="k8")
            v8 = big.tile([128, NPIX // 128, C], FP8, name="v8")
            kt8 = big.tile([128, NPIX // 128, C], FP8, name="kt8")
            G8 = big.tile([128, 2, C], FP8, name="G8")

            # ---- projection emitters (bf16, 1 cycle/row) ----
            def emit_k_slice(s, halves=1):
                t = stp.tile([128, 2, 512], F32, name="kproj", tag="st")
                for h in range(halves):
                    w = 512 // halves
                    lo = s * 512 + h * w
                    for o in range(2):
                        for i in range(2):
                            nc.tensor.matmul(
                                t[:, o, h * w : (h + 1) * w],
                                w_b["k"][i][:, o * 128 : (o + 1) * 128],
                                x_b[i][:, lo : lo + w],
                                start=(i == 0),
                                stop=(i == 1),
                            )
                # k bias is softmax-invariant; skip it
                nc.vector.tensor_copy(k8[:, :, s * 512 : (s + 1) * 512], t)

            q_first = [True]

            def emit_q_slice(s):
                t = stp.tile([128, 2, 512], F32, name="qproj", tag="st")
                for o in range(2):
                    for i in range(2):
                        nc.tensor.matmul(
                            t[:, o, :],
                            w_b["q"][i][:, o * 128 : (o + 1) * 128],
                            x_b[i][:, s * 512 : (s + 1) * 512],
                            start=(i == 0),
                            stop=(i == 1),
                        )
                sl = slice(s * 512, (s + 1) * 512)
                # q8 first (it gates the score matmuls); residual math after
                for o in range(2):
                    nc.vector.tensor_scalar(
                        q8[:, o, sl], t[:, o, :], 1.0, bq_sb[o], MULT, ADD
                    )
                if q_first[0]:
                    q_first[0] = False
                    for i in range(2):
                        nc.vector.tensor_scalar_mul(bq16_sb[i], bq_sb[i], 16.0)
                for o in range(2):
                    # tmp16 = 16*(q + bq); qr16 = tmp16 - 16*q8
                    tmp16 = small.tile([128, 512], F32, name="tmp16", tag="tmp16")
                    nc.vector.tensor_scalar(
                        tmp16, t[:, o, :], 16.0, bq16_sb[o], MULT, ADD
                    )
                    nc.vector.affine_then_add(
                        qr16[:, o, sl], q8[:, o, sl], tmp16, -16.0, 0.0
                    )

            def emit_v_pair(j):
                t = stp.tile([128, 2, 512], F32, name="vproj", tag="st")
                for h in range(2):
                    m = 2 * j + h
                    for i in range(2):
                        nc.tensor.matmul(
                            t[:, h, 0:C],
                            x_b[i][:, m * 128 : (m + 1) * 128],
                            w_b["v"][i],
                            start=(i == 0),
                            stop=(i == 1),
                        )
                # biasless v8 cast; alternate engines to balance DVE/Act load
                if j % 2 == 0:
                    nc.vector.tensor_copy(v8[:, 2 * j : 2 * j + 2, :], t[:, :, 0:C])
                else:
                    nc.scalar.activation(
                        v8[:, 2 * j : 2 * j + 2, :], t[:, :, 0:C], AF.Copy
                    )

            def emit_kt_pair(j):
                t = stp.tile([128, 2, 512], F32, name="ktproj", tag="st")
                for h in range(2):
                    m = 2 * j + h
                    for i in range(2):
                        nc.tensor.matmul(
                            t[:, h, 0:C],
                            x_b[i][:, m * 128 : (m + 1) * 128],
                            w_b["k"][i],
                            start=(i == 0),
                            stop=(i == 1),
                        )
                nc.vector.tensor_copy(kt8[:, 2 * j : 2 * j + 2, :], t[:, :, 0:C])

            # ---- G = (kt8^T v8)/16 accumulated in the spare psQ bank ----
            g_ps = {}

            def emit_g_matmul(half, j):
                if j == 0:
                    g_ps[half] = psQ.tile(
                        [128, C], F32, name=f"gps{half}", tag="rb"
                    )
                nc.tensor.matmul(
                    g_ps[half],
                    kt8[:, 2 * j : 2 * j + 2, half * 128 : (half + 1) * 128],
                    v8[:, 2 * j : 2 * j + 2, :],
                    start=(j == 0),
                    stop=(j == NJ - 1),
                    perf_mode=DR,
                )
                if j == NJ - 1:
                    nc.scalar.activation(
                        G8[:, half, :], g_ps[half], AF.Copy, scale=1.0 / 16.0
                    )

            # ---- main pipeline over (nt, j) steps ----
            pts = {}
            cur = {}
            pending_b = []  # (due_step, nt, u0, u1, rinv_r)
            last_corr = {}  # nt -> corr tile emitted early

            def emit_dn_corr(nt):
                # dn = qr16^T G8 slice; corr = dn/65536 + bv
                #     = (dq @ G)/4096 + bv   (bv rides through softmax intact)
                dn = stp.tile([128, 2, 512], F32, name="dn", tag="st")
                for ch in range(2):
                    nc.tensor.matmul(
                        dn[:, ch, :],
                        G8[:, :, ch * 128 : (ch + 1) * 128],
                        qr16[:, :, nt * 512 : (nt + 1) * 512],
                        start=True,
                        stop=True,
                        perf_mode=DR,
                    )
                corr = small.tile([128, 2, 512], BF16, name="corr", tag="corr")
                nc.scalar.activation(corr, dn, AF.Copy, scale=1.0 / 65536.0)
                return corr

            def emit_ot(nt, j):
                pt_t = pts.pop((nt, j))
                if j == 0:
                    ot0 = psO.tile([128, 512], F32, name="ot0", tag="ot0")
                    ot1 = psO.tile([128, 512], F32, name="ot1", tag="ot1")
                    rs = psR.tile([32, 512], F32, name="rs", tag="rs")
                    cur[nt] = (ot0, ot1, rs)
                ot0, ot1, rs = cur[nt]
                st_, sp_ = (j == 0), (j == NJ - 1)
                nc.tensor.matmul(
                    ot0, v8[:, 2 * j : 2 * j + 2, 0:128], pt_t,
                    start=st_, stop=sp_, perf_mode=DR,
                )
                nc.tensor.matmul(
                    ot1, v8[:, 2 * j : 2 * j + 2, 128:256], pt_t,
                    start=st_, stop=sp_, perf_mode=DR,
                )
                nc.tensor.matmul(
                    rs, ones8, pt_t, start=st_, stop=sp_, perf_mode=DR,
                )

            def emit_epilogue_a(step, nt):
                ot0, ot1, rs = cur.pop(nt)
                # nt 0/1 wait for G8 (ready after step ~2*NJ+2); later tiles
                # use the normal short deferral
                due = step + EPI_DELAY
                if nt <= 2:
                    # G8 half-1 finishes accumulating at step 3*NJ-1 (tile 2);
                    # stagger the three backed-up epilogues so their PE/Act/DVE
                    # bursts don't pile onto consecutive steps
                    due = max(due, 3 * NJ + 2 + 4 * nt)
                if nt == NT - 1:
                    # final tile: dn/corr were emitted early; run the rinv ->
                    # rb -> mul/add -> DMA chain in 256-column chunks so DVE,
                    # PE and the output DMA pipeline instead of serializing.
                    corr = last_corr.pop(nt)
                    for hh in range(2):
                        cs = slice(hh * 256, (hh + 1) * 256)
                        rinv_f = small.tile(
                            [1, 256], F32, name="rinv_f", tag="rinv_f"
                        )
                        nc.vector.reciprocal_approx_fast(rinv_f, rs[0:1, cs])
                        rinv_r = small.tile(
                            [1, 256], FP16, name="rinv_r", tag="rinv_r"
                        )
                        nc.vector.tensor_copy(rinv_r, rinv_f)
                        rb = psQ.tile([128, 256], F32, name="rbh", tag="rb")
                        nc.tensor.matmul(
                            rb, ones_row, rinv_r, start=True, stop=True
                        )
                        rb_sb = small.tile(
                            [128, 256], F32, name="rb_sb", tag="rb_sb"
                        )
                        nc.vector.tensor_copy(rb_sb, rb)
                        for oc, u in ((0, ot0), (1, ot1)):
                            osb = osbp.tile(
                                [128, 256], F32, name="osb", tag=f"osb{oc}"
                            )
                            nc.vector.tensor_mul(osb, u[:, cs], rb_sb)
                            osb2 = osbp.tile(
                                [128, 256], F32, name="osb2", tag=f"osb2{oc}"
                            )
                            nc.vector.scalar_tensor_tensor(
                                osb2, osb, bv_col[oc], corr[:, oc, cs], ADD, ADD
                            )
                            eng = nc.gpsimd if (hh + oc) % 2 else nc.sync
                            eng.dma_start(
                                out=out_d[
                                    oc * 128 : (oc + 1) * 128,
                                    nt * 512 + hh * 256 : nt * 512
                                    + (hh + 1) * 256,
                                ],
                                in_=osb2,
                            )
                    return
                u0 = up.tile([128, 512], F32, name="u0", tag="u0")
                nc.vector.tensor_copy(u0, ot0)
                u1 = up.tile([128, 512], F32, name="u1", tag="u1")
                nc.vector.tensor_copy(u1, ot1)
                rinv_f = small.tile([1, 512], F32, name="rinv_f", tag="rinv_f")
                nc.vector.reciprocal_approx_fast(rinv_f, rs[0:1, :])
                rinv_r = small.tile([1, 512], FP16, name="rinv_r", tag="rinv_r")
                nc.vector.tensor_copy(rinv_r, rinv_f)
                pending_b.append((due, nt, u0, u1, rinv_r))

            def emit_epilogue_b(nt, u0, u1, rinv_r):
                # (dn emitted before rb so the PE isn't gated on the rinv chain)
                corr = emit_dn_corr(nt)
                rb = psQ.tile([128, 512], F32, name="rb", tag="rb")
                nc.tensor.matmul(rb, ones_row, rinv_r, start=True, stop=True)
                for oc, u in ((0, u0), (1, u1)):
                    osb = osbp.tile([128, 512], F32, name="osb", tag=f"osb{oc}")
                    nc.vector.tensor_mul(osb, u, rb)
                    osb2 = osbp.tile([128, 512], F32, name="osb2", tag=f"osb2{oc}")
                    nc.vector.scalar_tensor_tensor(
                        osb2, osb, bv_col[oc], corr[:, oc, :], ADD, ADD
                    )
                    nc.sync.dma_start(
                        out=out_d[
                            oc * 128 : (oc + 1) * 128, nt * 512 : (nt + 1) * 512
                        ],
                        in_=osb2,
                    )

            k_emitted = 0
            q_emitted = 0
            TOTAL = NT * NJ
            for step in range(TOTAL + LAG):
                while pending_b and pending_b[0][0] <= step:
                    _, nt_, u0_, u1_, rv_ = pending_b.pop(0)
                    emit_epilogue_b(nt_, u0_, u1_, rv_)

                if step < TOTAL:
                    nt, j = divmod(step, NJ)
                    if nt == 0:
                        need_k = min(NT, (2 * j + 1) // 4 + 2)
                        while k_emitted < need_k:
                            emit_k_slice(k_emitted, halves=2 if k_emitted == 0 else 1)
                            k_emitted += 1
                        if q_emitted == 0:
                            emit_q_slice(0)
                            q_emitted = 1
                        if j == 1:
                            emit_ones8()
                        elif j == 2:
                            emit_ones_row()
                    if j == 8 and nt < NT - 1:
                        emit_q_slice(nt + 1)
                        q_emitted += 1
                    # scores (fp8 DoubleRow) + exp
                    st_t = stp.tile([128, 2, 512], F32, name="st_t", tag="st")
                    for h in range(2):
                        m = 2 * j + h
                        nc.tensor.matmul(
                            st_t[:, h, :],
                            k8[:, :, m * 128 : (m + 1) * 128],
                            q8[:, :, nt * 512 : (nt + 1) * 512],
                            start=True,
                            stop=True,
                            perf_mode=DR,
                        )
                    pt_t = ptp.tile([128, 2, 512], FP8, name="pt_t", tag="pt")
                    nc.scalar.activation(pt_t, st_t, AF.Exp, scale=SCALE)
                    pts[(nt, j)] = pt_t
                    if nt == 0:
                        emit_v_pair(j)
                    elif nt == 1:
                        # pixels-major K proj + G accumulation ride tile 1
                        emit_kt_pair(j)
                        emit_g_matmul(0, j)
                    elif nt == 2:
                        emit_g_matmul(1, j)
                    elif nt == NT - 1 and j == 2:
                        # final tile's correction, computed well before the
                        # tail so the last epilogue is just rinv/rb/mul/DMA
                        last_corr[nt] = emit_dn_corr(nt)

                if step >= LAG:
                    nt2, j2 = divmod(step - LAG, NJ)
                    emit_ot(nt2, j2)
                    if j2 == NJ - 1:
                        emit_epilogue_a(step, nt2)

            while pending_b:
                _, nt_, u0_, u1_, rv_ = pending_b.pop(0)
                emit_epilogue_b(nt_, u0_, u1_, rv_)

    nc.compile()
    return nc


def _get_nc():
    if "nc" not in _CACHE:
        _CACHE["nc"] = _build()
    return _CACHE["nc"]


def make_in_maps(x, wq, wk, wv, bq, bv):
    bf = ml_dtypes.bfloat16
    shared = {
        "wq_t": np.ascontiguousarray(wq.T.astype(bf)),
        "wk_t": np.ascontiguousarray(wk.T.astype(bf)),
        "wv_t": np.ascontiguousarray(wv.T.astype(bf)),
        "bq": np.ascontiguousarray(bq.reshape(C, 1).astype(np.float32)),
        "bv": np.ascontiguousarray(bv.reshape(1, C).astype(np.float32)),
    }
    return [
        {
            "x": np.ascontiguousarray(x[b].reshape(C, NPIX).astype(bf)),
            **shared,
        }
        for b in range(B)
    ]


def kernel(x, wq, wk, wv, bq, bk, bv):
    x = np.asarray(x, dtype=np.float32)
    wq = np.asarray(wq, dtype=np.float32)
    wk = np.asarray(wk, dtype=np.float32)
    wv = np.asarray(wv, dtype=np.float32)
    bq = np.asarray(bq, dtype=np.float32)
    bv = np.asarray(bv, dtype=np.float32)

    nc = _get_nc()
    in_maps = make_in_maps(x, wq, wk, wv, bq, bv)
    res = run_bass_kernel_spmd(nc, in_maps, core_ids=list(range(B)))
    out = np.stack([res.results[b]["out"] for b in range(B)])
    return out.reshape(B, C, 64, 64)


# revision 16
# speedup vs baseline: 1.0108x; 1.0108x over previous
"""Trainium2 Bass kernel for per-batch spatial self-attention — fp8 DoubleRow.

Per-core computation (one batch image per NeuronCore, 8 cores):
  x:(256, 4096) bf16 -> q/k = W x + b (channels-major, fp8e4m3 + residual qr16)
                        v = x^T W^T + b (pixels-major, fp8e4m3)
  St[m,n] = sum_c K8[c,m] Q8[c,n]  fp8 DoubleRow (256-deep contraction/instr)
  Pt = exp(St/16) -> fp8           one Act instruction per 2-bank PSUM tile
  OT[o,n] = sum_m V[m,o] Pt[m,n]   fp8 DoubleRow, key pairs
  rowsum via fp8 DoubleRow ones-matmul accumulated across key pairs
  out = OT/rowsum + (dq @ G)/N     Gram correction: G = K^T V / 16 cancels the
                                   query-side fp8 quantization error.
  qr16 = fp8(16*(q - q8)) is the stored q-residual; G accumulates in the spare
  PSUM bank during tiles 0-1 from a pixels-major K projection (kt8) and v8.

Engine placement: projections run in bf16 on the PE (1 cycle/row); exp and the
correction scale on Act; q8/k8 quantization and the normalize chain on DVE;
v8/kt8/u PSUM->SBUF moves on gpsimd (idle otherwise — all input DMAs ride the
hw DGE sync queue). x and the conv weights are pre-cast to bf16 on the host.
"""

import sys

sys.path.insert(0, "/opt/trn_rl_repo")

import numpy as np
import ml_dtypes
import concourse.bacc as bacc
import concourse.bass as bass
import concourse.mybir as mybir
import concourse.tile as tile
from concourse.bass_utils import run_bass_kernel_spmd

F32 = mybir.dt.float32
FP8 = mybir.dt.float8e4
BF16 = mybir.dt.bfloat16
FP16 = mybir.dt.float16
AF = mybir.ActivationFunctionType
DR = mybir.MatmulPerfMode.DoubleRow
MULT = mybir.AluOpType.mult
ADD = mybir.AluOpType.add

B = 8
C = 256  # channels
NPIX = 4096  # 64*64
NT = NPIX // 512  # 8 query tiles of 512
NJ = NPIX // 256  # 16 key-chunk pairs (2x128) per query tile
SCALE = 1.0 / 16.0  # 1/sqrt(C)
LAG = 4  # OT stream lags the score stream by this many (nt,j) steps
EPI_DELAY = 2  # epilogue part B deferred this many steps (G-gated for nt 0/1)

_CACHE = {}


def _build():
    nc = bacc.Bacc("TRN2", num_swdge_queues=4)
    x_d = nc.declare_dram_parameter("x", [C, NPIX], BF16, isOutput=False)
    wq_d = nc.declare_dram_parameter("wq_t", [C, C], BF16, isOutput=False)
    wk_d = nc.declare_dram_parameter("wk_t", [C, C], BF16, isOutput=False)
    wv_d = nc.declare_dram_parameter("wv_t", [C, C], BF16, isOutput=False)
    bq_d = nc.declare_dram_parameter("bq", [C, 1], F32, isOutput=False)
    bv_d = nc.declare_dram_parameter("bv", [1, C], F32, isOutput=False)
    out_d = nc.declare_dram_parameter("out", [C, NPIX], F32, isOutput=True)

    with tile.TileContext(nc) as tc:
        with (
            tc.tile_pool(name="big", bufs=1) as big,
            tc.tile_pool(name="small", bufs=2) as small,
            tc.tile_pool(name="ptp", bufs=LAG + 3) as ptp,
            tc.tile_pool(name="up", bufs=3) as up,
            tc.tile_pool(name="osbp", bufs=2) as osbp,
            tc.tile_pool(name="stp", bufs=2, space="PSUM") as stp,
            tc.tile_pool(name="psO", bufs=1, space="PSUM") as psO,
            tc.tile_pool(name="psR", bufs=1, space="PSUM") as psR,
            tc.tile_pool(name="psQ", bufs=1, space="PSUM") as psQ,
        ):
            # ---- warm-up tiles first: their DVE ops have no DMA deps, so
            # the PE p-state ramp starts immediately ----
            warm_f = small.tile([128, 256], F32, name="warm_f", tag="warm_f")
            nc.vector.memset(warm_f, 1.0)
            warm_r = small.tile([128, 256], BF16, name="warm_r", tag="warm_r")
            nc.vector.tensor_copy(warm_r, warm_f)
            warm_c = small.tile([128, 1], BF16, name="warm_c", tag="warm_c")
            nc.vector.tensor_copy(warm_c, warm_f[:, 0:1])
            warm_ps = stp.tile([128, 2, 512], F32, name="warm_ps", tag="st")
            for _ in range(12):
                nc.tensor.matmul(
                    warm_ps[0:1, 0, 0:256],
                    warm_c,
                    warm_r,
                    start=True,
                    stop=True,
                    skip_group_check=True,
                )

            # ---- input DMAs on the hw DGE (sync) queue, need-ordered;
            # the gpsimd/swdge queues stay free for compute offload ----
            w_b = {}
            for nm in ("q", "k", "v"):
                w_b[nm] = [
                    big.tile([128, C], BF16, name=f"w{nm}_b{i}") for i in range(2)
                ]
            x_b = [big.tile([128, NPIX], BF16, name=f"x_b{i}") for i in range(2)]

            wd = {"q": wq_d, "k": wk_d, "v": wv_d}
            for nm in ("k", "q", "v"):
                for i in range(2):
                    nc.sync.dma_start(
                        out=w_b[nm][i], in_=wd[nm][i * 128 : (i + 1) * 128, :]
                    )
            bq_sb = [big.tile([128, 1], F32, name=f"bq_sb{i}") for i in range(2)]
            for i in range(2):
                nc.sync.dma_start(out=bq_sb[i], in_=bq_d[i * 128 : (i + 1) * 128, :])
            # bv as per-partition columns (added via the corr Act bias: the
            # V-bias passes through row-normalized attention unchanged)
            bv_col = [big.tile([128, 1], F32, name=f"bv_col{i}") for i in range(2)]
            for i in range(2):
                bv_col_ap = bass.AP(
                    tensor=bv_d.ap().tensor,
                    offset=i * 128,
                    ap=[[1, 128], [0, 1]],
                )
                nc.sync.dma_start(out=bv_col[i], in_=bv_col_ap)
            # x rides the 4 swdge queues (parallel dispatch; the sync SEQ
            # engine issues DMA_DIRECT2D too slowly for 20 need-ordered chunks)
            for half in range(4):
                for i in range(2):
                    lo, hi = half * 256, (half + 1) * 256
                    nc.gpsimd.dma_start(
                        out=x_b[i][:, lo:hi], in_=x_d[i * 128 : (i + 1) * 128, lo:hi]
                    )
            for s in range(2, NT):
                lo, hi = s * 512, (s + 1) * 512
                for i in range(2):
                    nc.gpsimd.dma_start(
                        out=x_b[i][:, lo:hi], in_=x_d[i * 128 : (i + 1) * 128, lo:hi]
                    )

            bq16_sb = [big.tile([128, 1], F32, name=f"bq16_sb{i}") for i in range(2)]

            # constants (DVE ops emitted lazily inside the step loop so the
            # first k8/q8 casts aren't queued behind them)
            ones_f2 = big.tile([128, 2, 32], F32, name="ones_f2")
            ones8 = big.tile([128, 2, 32], FP8, name="ones8")
            ones_rf = big.tile([1, 128], F32, name="ones_rf")
            ones_row = big.tile([1, 128], FP16, name="ones_row")

            def emit_ones8():
                nc.vector.memset(ones_f2, 1.0)
                nc.vector.tensor_copy(ones8, ones_f2)

            def emit_ones_row():
                nc.vector.memset(ones_rf, 1.0)
                nc.vector.tensor_copy(ones_row, ones_rf)

            # fp8 storage
            q8 = big.tile([128, 2, NPIX], FP8, name="q8")
            qr16 = big.tile([128, 2, NPIX], FP8, name="qr16")
            k8 = big.tile([128, 2, NPIX], FP8, name="k8")
            v8 = big.tile([128, NPIX // 128, C], FP8, name="v8")
            kt8 = big.tile([128, NPIX // 128, C], FP8, name="kt8")
            G8 = big.tile([128, 2, C], FP8, name="G8")

            # ---- projection emitters (bf16, 1 cycle/row) ----
            def emit_k_slice(s, halves=1):
                t = stp.tile([128, 2, 512], F32, name="kproj", tag="st")
                for h in range(halves):
                    w = 512 // halves
                    lo = s * 512 + h * w
                    for o in range(2):
                        for i in range(2):
                            nc.tensor.matmul(
                                t[:, o, h * w : (h + 1) * w],
                                w_b["k"][i][:, o * 128 : (o + 1) * 128],
                                x_b[i][:, lo : lo + w],
                                start=(i == 0),
                                stop=(i == 1),
                            )
                # k bias is softmax-invariant; skip it
                nc.vector.tensor_copy(k8[:, :, s * 512 : (s + 1) * 512], t)

            q_first = [True]

            def emit_q_slice(s):
                t = stp.tile([128, 2, 512], F32, name="qproj", tag="st")
                for o in range(2):
                    for i in range(2):
                        nc.tensor.matmul(
                            t[:, o, :],
                            w_b["q"][i][:, o * 128 : (o + 1) * 128],
                            x_b[i][:, s * 512 : (s + 1) * 512],
                            start=(i == 0),
                            stop=(i == 1),
                        )
                sl = slice(s * 512, (s + 1) * 512)
                # q8 first (it gates the score matmuls); residual math after
                for o in range(2):
                    nc.vector.tensor_scalar(
                        q8[:, o, sl], t[:, o, :], 1.0, bq_sb[o], MULT, ADD
                    )
                if q_first[0]:
                    q_first[0] = False
                    for i in range(2):
                        nc.vector.tensor_scalar_mul(bq16_sb[i], bq_sb[i], 16.0)
                for o in range(2):
                    # tmp16 = 16*(q + bq); qr16 = tmp16 - 16*q8
                    tmp16 = small.tile([128, 512], F32, name="tmp16", tag="tmp16")
                    nc.vector.tensor_scalar(
                        tmp16, t[:, o, :], 16.0, bq16_sb[o], MULT, ADD
                    )
                    nc.vector.affine_then_add(
                        qr16[:, o, sl], q8[:, o, sl], tmp16, -16.0, 0.0
                    )

            def emit_v_pair(j):
                t = stp.tile([128, 2, 512], F32, name="vproj", tag="st")
                for h in range(2):
                    m = 2 * j + h
                    for i in range(2):
                        nc.tensor.matmul(
                            t[:, h, 0:C],
                            x_b[i][:, m * 128 : (m + 1) * 128],
                            w_b["v"][i],
                            start=(i == 0),
                            stop=(i == 1),
                        )
                # biasless v8 cast; alternate engines to balance DVE/Act load
                if j % 2 == 0:
                    nc.vector.tensor_copy(v8[:, 2 * j : 2 * j + 2, :], t[:, :, 0:C])
                else:
                    nc.scalar.activation(
                        v8[:, 2 * j : 2 * j + 2, :], t[:, :, 0:C], AF.Copy
                    )

            def emit_kt_pair(j):
                t = stp.tile([128, 2, 512], F32, name="ktproj", tag="st")
                for h in range(2):
                    m = 2 * j + h
                    for i in range(2):
                        nc.tensor.matmul(
                            t[:, h, 0:C],
                            x_b[i][:, m * 128 : (m + 1) * 128],
                            w_b["k"][i],
                            start=(i == 0),
                            stop=(i == 1),
                        )
                nc.vector.tensor_copy(kt8[:, 2 * j : 2 * j + 2, :], t[:, :, 0:C])

            # ---- G = (kt8^T v8)/16 accumulated in the spare psQ bank ----
            g_ps = {}

            def emit_g_matmul(half, j):
                if j == 0:
                    g_ps[half] = psQ.tile(
                        [128, C], F32, name=f"gps{half}", tag="rb"
                    )
                nc.tensor.matmul(
                    g_ps[half],
                    kt8[:, 2 * j : 2 * j + 2, half * 128 : (half + 1) * 128],
                    v8[:, 2 * j : 2 * j + 2, :],
                    start=(j == 0),
                    stop=(j == NJ - 1),
                    perf_mode=DR,
                )
                if j == NJ - 1:
                    nc.scalar.activation(
                        G8[:, half, :], g_ps[half], AF.Copy, scale=1.0 / 16.0
                    )

            # ---- main pipeline over (nt, j) steps ----
            pts = {}
            cur = {}
            pending_b = []  # (due_step, nt, u0, u1, rinv_r)
            last_corr = {}  # nt -> corr tile emitted early

            def emit_dn_corr(nt):
                # dn = qr16^T G8 slice; corr = dn/65536 + bv
                #     = (dq @ G)/4096 + bv   (bv rides through softmax intact)
                dn = stp.tile([128, 2, 512], F32, name="dn", tag="st")
                for ch in range(2):
                    nc.tensor.matmul(
                        dn[:, ch, :],
                        G8[:, :, ch * 128 : (ch + 1) * 128],
                        qr16[:, :, nt * 512 : (nt + 1) * 512],
                        start=True,
                        stop=True,
                        perf_mode=DR,
                    )
                # corr = dn/65536 + bv on the DVE (Act is exp-saturated in
                # steady state; bv rides through row-normalized attention)
                corr = small.tile([128, 2, 512], F32, name="corr", tag="corr")
                for ch in range(2):
                    nc.vector.tensor_scalar(
                        corr[:, ch, :],
                        dn[:, ch, :],
                        1.0 / 65536.0,
                        bv_col[ch],
                        MULT,
                        ADD,
                    )
                return corr

            def emit_ot(nt, j):
                pt_t = pts.pop((nt, j))
                if j == 0:
                    ot0 = psO.tile([128, 512], F32, name="ot0", tag="ot0")
                    ot1 = psO.tile([128, 512], F32, name="ot1", tag="ot1")
                    rs = psR.tile([32, 512], F32, name="rs", tag="rs")
                    cur[nt] = (ot0, ot1, rs)
                ot0, ot1, rs = cur[nt]
                st_, sp_ = (j == 0), (j == NJ - 1)
                nc.tensor.matmul(
                    ot0, v8[:, 2 * j : 2 * j + 2, 0:128], pt_t,
                    start=st_, stop=sp_, perf_mode=DR,
                )
                nc.tensor.matmul(
                    ot1, v8[:, 2 * j : 2 * j + 2, 128:256], pt_t,
                    start=st_, stop=sp_, perf_mode=DR,
                )
                nc.tensor.matmul(
                    rs, ones8, pt_t, start=st_, stop=sp_, perf_mode=DR,
                )

            def emit_epilogue_a(step, nt):
                ot0, ot1, rs = cur.pop(nt)
                # nt 0/1 wait for G8 (ready after step ~2*NJ+2); later tiles
                # use the normal short deferral
                due = step + EPI_DELAY
                if nt <= 2:
                    # G8 half-1 finishes accumulating at step 3*NJ-1 (tile 2);
                    # stagger the three backed-up epilogues so their PE/Act/DVE
                    # bursts don't pile onto consecutive steps
                    due = max(due, 3 * NJ + 2 + 4 * nt)
                if nt == NT - 1:
                    # final tile: dn/corr were emitted early; run the rinv ->
                    # rb -> mul/add -> DMA chain in 256-column chunks so DVE,
                    # PE and the output DMA pipeline instead of serializing.
                    corr = last_corr.pop(nt)
                    for hh in range(2):
                        cs = slice(hh * 256, (hh + 1) * 256)
                        rinv_f = small.tile(
                            [1, 256], F32, name="rinv_f", tag="rinv_f"
                        )
                        nc.vector.reciprocal_approx_fast(rinv_f, rs[0:1, cs])
                        rinv_r = small.tile(
                            [1, 256], FP16, name="rinv_r", tag="rinv_r"
                        )
                        nc.vector.tensor_copy(rinv_r, rinv_f)
                        rb = psQ.tile([128, 256], F32, name="rbh", tag="rb")
                        nc.tensor.matmul(
                            rb, ones_row, rinv_r, start=True, stop=True
                        )
                        rb_sb = small.tile(
                            [128, 256], F32, name="rb_sb", tag="rb_sb"
                        )
                        nc.vector.tensor_copy(rb_sb, rb)
                        for oc, u in ((0, ot0), (1, ot1)):
                            osb = osbp.tile(
                                [128, 256], F32, name="osb", tag=f"osb{oc}"
                            )
                            nc.vector.tensor_mul(osb, u[:, cs], rb_sb)
                            osb2 = osbp.tile(
                                [128, 256], F32, name="osb2", tag=f"osb2{oc}"
                            )
                            nc.vector.tensor_add(osb2, osb, corr[:, oc, cs])
                            eng = nc.gpsimd if (hh + oc) % 2 else nc.sync
                            eng.dma_start(
                                out=out_d[
                                    oc * 128 : (oc + 1) * 128,
                                    nt * 512 + hh * 256 : nt * 512
                                    + (hh + 1) * 256,
                                ],
                                in_=osb2,
                            )
                    return
                u0 = up.tile([128, 512], F32, name="u0", tag="u0")
                nc.vector.tensor_copy(u0, ot0)
                u1 = up.tile([128, 512], F32, name="u1", tag="u1")
                nc.vector.tensor_copy(u1, ot1)
                rinv_f = small.tile([1, 512], F32, name="rinv_f", tag="rinv_f")
                nc.vector.reciprocal_approx_fast(rinv_f, rs[0:1, :])
                rinv_r = small.tile([1, 512], FP16, name="rinv_r", tag="rinv_r")
                nc.vector.tensor_copy(rinv_r, rinv_f)
                pending_b.append((due, nt, u0, u1, rinv_r))

            def emit_epilogue_b(nt, u0, u1, rinv_r):
                # (dn emitted before rb so the PE isn't gated on the rinv chain)
                corr = emit_dn_corr(nt)
                rb = psQ.tile([128, 512], F32, name="rb", tag="rb")
                nc.tensor.matmul(rb, ones_row, rinv_r, start=True, stop=True)
                for oc, u in ((0, u0), (1, u1)):
                    osb = osbp.tile([128, 512], F32, name="osb", tag=f"osb{oc}")
                    nc.vector.tensor_mul(osb, u, rb)
                    osb2 = osbp.tile([128, 512], F32, name="osb2", tag=f"osb2{oc}")
                    nc.vector.tensor_add(osb2, osb, corr[:, oc, :])
                    nc.sync.dma_start(
                        out=out_d[
                            oc * 128 : (oc + 1) * 128, nt * 512 : (nt + 1) * 512
                        ],
                        in_=osb2,
                    )

            k_emitted = 0
            q_emitted = 0
            TOTAL = NT * NJ
            for step in range(TOTAL + LAG):
                while pending_b and pending_b[0][0] <= step:
                    _, nt_, u0_, u1_, rv_ = pending_b.pop(0)
                    emit_epilogue_b(nt_, u0_, u1_, rv_)

                if step < TOTAL:
                    nt, j = divmod(step, NJ)
                    if nt == 0:
                        need_k = min(NT, (2 * j + 1) // 4 + 2)
                        while k_emitted < need_k:
                            emit_k_slice(k_emitted, halves=2 if k_emitted == 0 else 1)
                            k_emitted += 1
                        if q_emitted == 0:
                            emit_q_slice(0)
                            q_emitted = 1
                        if j == 1:
                            emit_ones8()
                        elif j == 2:
                            emit_ones_row()
                    if j == 8 and nt < NT - 1:
                        emit_q_slice(nt + 1)
                        q_emitted += 1
                    # scores (fp8 DoubleRow) + exp
                    st_t = stp.tile([128, 2, 512], F32, name="st_t", tag="st")
                    for h in range(2):
                        m = 2 * j + h
                        nc.tensor.matmul(
                            st_t[:, h, :],
                            k8[:, :, m * 128 : (m + 1) * 128],
                            q8[:, :, nt * 512 : (nt + 1) * 512],
                            start=True,
                            stop=True,
                            perf_mode=DR,
                        )
                    pt_t = ptp.tile([128, 2, 512], FP8, name="pt_t", tag="pt")
                    nc.scalar.activation(pt_t, st_t, AF.Exp, scale=SCALE)
                    pts[(nt, j)] = pt_t
                    if nt == 0:
                        emit_v_pair(j)
                    elif nt == 1:
                        # pixels-major K proj + G accumulation ride tile 1
                        emit_kt_pair(j)
                        emit_g_matmul(0, j)
                    elif nt == 2:
                        emit_g_matmul(1, j)
                    elif nt == NT - 1 and j == 2:
                        # final tile's correction, computed well before the
                        # tail so the last epilogue is just rinv/rb/mul/DMA
                        last_corr[nt] = emit_dn_corr(nt)

                if step >= LAG:
                    nt2, j2 = divmod(step - LAG, NJ)
                    emit_ot(nt2, j2)
                    if j2 == NJ - 1:
                        emit_epilogue_a(step, nt2)

            while pending_b:
                _, nt_, u0_, u1_, rv_ = pending_b.pop(0)
                emit_epilogue_b(nt_, u0_, u1_, rv_)

    nc.compile()
    return nc


def _get_nc():
    if "nc" not in _CACHE:
        _CACHE["nc"] = _build()
    return _CACHE["nc"]


def make_in_maps(x, wq, wk, wv, bq, bv):
    bf = ml_dtypes.bfloat16
    shared = {
        "wq_t": np.ascontiguousarray(wq.T.astype(bf)),
        "wk_t": np.ascontiguousarray(wk.T.astype(bf)),
        "wv_t": np.ascontiguousarray(wv.T.astype(bf)),
        "bq": np.ascontiguousarray(bq.reshape(C, 1).astype(np.float32)),
        "bv": np.ascontiguousarray(bv.reshape(1, C).astype(np.float32)),
    }
    return [
        {
            "x": np.ascontiguousarray(x[b].reshape(C, NPIX).astype(bf)),
            **shared,
        }
        for b in range(B)
    ]


def kernel(x, wq, wk, wv, bq, bk, bv):
    x = np.asarray(x, dtype=np.float32)
    wq = np.asarray(wq, dtype=np.float32)
    wk = np.asarray(wk, dtype=np.float32)
    wv = np.asarray(wv, dtype=np.float32)
    bq = np.asarray(bq, dtype=np.float32)
    bv = np.asarray(bv, dtype=np.float32)

    nc = _get_nc()
    in_maps = make_in_maps(x, wq, wk, wv, bq, bv)
    res = run_bass_kernel_spmd(nc, in_maps, core_ids=list(range(B)))
    out = np.stack([res.results[b]["out"] for b in range(B)])
    return out.reshape(B, C, 64, 64)


# revision 19
# speedup vs baseline: 4.9108x; 4.8583x over previous
"""Trainium2 Bass kernel for per-batch spatial self-attention — fp8 DoubleRow.

Per-core computation (one batch image per NeuronCore, 8 cores):
  x:(256, 4096) bf16 -> q/k = W x + b (channels-major, fp8e4m3 + residual qr16)
                        v = x^T W^T + b (pixels-major, fp8e4m3)
  St[m,n] = sum_c K8[c,m] Q8[c,n]  fp8 DoubleRow (256-deep contraction/instr)
  Pt = exp(St/16) -> fp8           one Act instruction per 2-bank PSUM tile
  OT[o,n] = sum_m V[m,o] Pt[m,n]   fp8 DoubleRow, key pairs
  rowsum via fp8 DoubleRow ones-matmul accumulated across key pairs
  out = OT/rowsum + (dq @ G)/N     Gram correction: G = K^T V / 16 cancels the
                                   query-side fp8 quantization error.
  qr16 = fp8(16*(q - q8)) is the stored q-residual; G accumulates in the spare
  PSUM bank during tiles 0-1 from a pixels-major K projection (kt8) and v8.

Engine placement: projections run in bf16 on the PE (1 cycle/row); exp and the
correction scale on Act; q8/k8 quantization and the normalize chain on DVE;
v8/kt8/u PSUM->SBUF moves on gpsimd (idle otherwise — all input DMAs ride the
hw DGE sync queue). x and the conv weights are pre-cast to bf16 on the host.
"""

import sys

sys.path.insert(0, "/opt/trn_rl_repo")

import numpy as np
import ml_dtypes
import concourse.bacc as bacc
import concourse.bass as bass
import concourse.mybir as mybir
import concourse.tile as tile
from concourse.bass_utils import run_bass_kernel_spmd

F32 = mybir.dt.float32
FP8 = mybir.dt.float8e4
BF16 = mybir.dt.bfloat16
FP16 = mybir.dt.float16
AF = mybir.ActivationFunctionType
DR = mybir.MatmulPerfMode.DoubleRow
MULT = mybir.AluOpType.mult
ADD = mybir.AluOpType.add

B = 8
C = 256  # channels
NPIX = 4096  # 64*64
NT = NPIX // 512  # 8 query tiles of 512
NJ = NPIX // 256  # 16 key-chunk pairs (2x128) per query tile
SCALE = 1.0 / 16.0  # 1/sqrt(C)
LAG = 4  # OT stream lags the score stream by this many (nt,j) steps
EPI_DELAY = 2  # epilogue part B deferred this many steps (G-gated for nt 0/1)

_CACHE = {}


def _build():
    nc = bacc.Bacc("TRN2", num_swdge_queues=4)
    x_d = nc.declare_dram_parameter("x", [C, NPIX], BF16, isOutput=False)
    wq_d = nc.declare_dram_parameter("wq_t", [C, C], BF16, isOutput=False)
    wk_d = nc.declare_dram_parameter("wk_t", [C, C], BF16, isOutput=False)
    wv_d = nc.declare_dram_parameter("wv_t", [C, C], BF16, isOutput=False)
    bq_d = nc.declare_dram_parameter("bq", [C, 1], F32, isOutput=False)
    bv_d = nc.declare_dram_parameter("bv", [1, C], F32, isOutput=False)
    out_d = nc.declare_dram_parameter("out", [C, NPIX], F32, isOutput=True)

    with tile.TileContext(nc) as tc:
        with (
            tc.tile_pool(name="big", bufs=1) as big,
            tc.tile_pool(name="small", bufs=2) as small,
            tc.tile_pool(name="ptp", bufs=LAG + 3) as ptp,
            tc.tile_pool(name="up", bufs=3) as up,
            tc.tile_pool(name="osbp", bufs=2) as osbp,
            tc.tile_pool(name="stp", bufs=2, space="PSUM") as stp,
            tc.tile_pool(name="psO", bufs=1, space="PSUM") as psO,
            tc.tile_pool(name="psR", bufs=1, space="PSUM") as psR,
            tc.tile_pool(name="psQ", bufs=1, space="PSUM") as psQ,
        ):
            # ---- warm-up tiles first: their DVE ops have no DMA deps, so
            # the PE p-state ramp starts immediately ----
            warm_f = small.tile([128, 256], F32, name="warm_f", tag="warm_f")
            nc.vector.memset(warm_f, 1.0)
            warm_r = small.tile([128, 256], BF16, name="warm_r", tag="warm_r")
            nc.vector.tensor_copy(warm_r, warm_f)
            warm_c = small.tile([128, 1], BF16, name="warm_c", tag="warm_c")
            nc.vector.tensor_copy(warm_c, warm_f[:, 0:1])
            warm_ps = stp.tile([128, 2, 512], F32, name="warm_ps", tag="st")
            for _ in range(12):
                nc.tensor.matmul(
                    warm_ps[0:1, 0, 0:256],
                    warm_c,
                    warm_r,
                    start=True,
                    stop=True,
                    skip_group_check=True,
                )

            # ---- input DMAs on the hw DGE (sync) queue, need-ordered;
            # the gpsimd/swdge queues stay free for compute offload ----
            w_b = {}
            for nm in ("q", "k", "v"):
                w_b[nm] = [
                    big.tile([128, C], BF16, name=f"w{nm}_b{i}") for i in range(2)
                ]
            x_b = [big.tile([128, NPIX], BF16, name=f"x_b{i}") for i in range(2)]

            wd = {"q": wq_d, "k": wk_d, "v": wv_d}
            for nm in ("k", "q", "v"):
                for i in range(2):
                    nc.sync.dma_start(
                        out=w_b[nm][i], in_=wd[nm][i * 128 : (i + 1) * 128, :]
                    )
            bq_sb = [big.tile([128, 1], F32, name=f"bq_sb{i}") for i in range(2)]
            for i in range(2):
                nc.sync.dma_start(out=bq_sb[i], in_=bq_d[i * 128 : (i + 1) * 128, :])
            # bv as per-partition columns (added via the corr Act bias: the
            # V-bias passes through row-normalized attention unchanged)
            bv_col = [big.tile([128, 1], F32, name=f"bv_col{i}") for i in range(2)]
            for i in range(2):
                bv_col_ap = bass.AP(
                    tensor=bv_d.ap().tensor,
                    offset=i * 128,
                    ap=[[1, 128], [0, 1]],
                )
                nc.sync.dma_start(out=bv_col[i], in_=bv_col_ap)
            # x rides the 4 swdge queues (parallel dispatch; the sync SEQ
            # engine issues DMA_DIRECT2D too slowly for 20 need-ordered chunks)
            for half in range(4):
                for i in range(2):
                    lo, hi = half * 256, (half + 1) * 256
                    nc.gpsimd.dma_start(
                        out=x_b[i][:, lo:hi], in_=x_d[i * 128 : (i + 1) * 128, lo:hi]
                    )
            for s in range(2, NT):
                lo, hi = s * 512, (s + 1) * 512
                for i in range(2):
                    nc.gpsimd.dma_start(
                        out=x_b[i][:, lo:hi], in_=x_d[i * 128 : (i + 1) * 128, lo:hi]
                    )

            bq16_sb = [big.tile([128, 1], F32, name=f"bq16_sb{i}") for i in range(2)]

            # constants (DVE ops emitted lazily inside the step loop so the
            # first k8/q8 casts aren't queued behind them)
            ones_f2 = big.tile([128, 2, 32], F32, name="ones_f2")
            ones8 = big.tile([128, 2, 32], FP8, name="ones8")
            ones_rf = big.tile([1, 128], F32, name="ones_rf")
            ones_row = big.tile([1, 128], FP16, name="ones_row")

            def emit_ones8():
                nc.vector.memset(ones_f2, 1.0)
                nc.vector.tensor_copy(ones8, ones_f2)

            def emit_ones_row():
                nc.vector.memset(ones_rf, 1.0)
                nc.vector.tensor_copy(ones_row, ones_rf)

            # fp8 storage
            q8 = big.tile([128, 2, NPIX], FP8, name="q8")
            qr16 = big.tile([128, 2, NPIX], FP8, name="qr16")
            k8 = big.tile([128, 2, NPIX], FP8, name="k8")
            v8 = big.tile([128, NPIX // 128, C], FP8, name="v8")
            kt8 = big.tile([128, NPIX // 128, C], FP8, name="kt8")
            G8 = big.tile([128, 2, C], FP8, name="G8")

            # ---- projection emitters (bf16, 1 cycle/row) ----
            def emit_k_slice(s, halves=1):
                t = stp.tile([128, 2, 512], F32, name="kproj", tag="st")
                for h in range(halves):
                    w = 512 // halves
                    lo = s * 512 + h * w
                    for o in range(2):
                        for i in range(2):
                            nc.tensor.matmul(
                                t[:, o, h * w : (h + 1) * w],
                                w_b["k"][i][:, o * 128 : (o + 1) * 128],
                                x_b[i][:, lo : lo + w],
                                start=(i == 0),
                                stop=(i == 1),
                            )
                # k bias is softmax-invariant; skip it
                nc.vector.tensor_copy(k8[:, :, s * 512 : (s + 1) * 512], t)

            q_first = [True]
            pending_qr = {}  # s -> (tmp16_o0, tmp16_o1)

            def emit_q_slice(s):
                t = stp.tile([128, 2, 512], F32, name="qproj", tag="st")
                for o in range(2):
                    for i in range(2):
                        nc.tensor.matmul(
                            t[:, o, :],
                            w_b["q"][i][:, o * 128 : (o + 1) * 128],
                            x_b[i][:, s * 512 : (s + 1) * 512],
                            start=(i == 0),
                            stop=(i == 1),
                        )
                sl = slice(s * 512, (s + 1) * 512)
                # q8 first (it gates the score matmuls)
                for o in range(2):
                    nc.vector.tensor_scalar(
                        q8[:, o, sl], t[:, o, :], 1.0, bq_sb[o], MULT, ADD
                    )
                if q_first[0]:
                    q_first[0] = False
                    for i in range(2):
                        nc.vector.tensor_scalar_mul(bq16_sb[i], bq_sb[i], 16.0)
                # tmp16 = 16*(q + bq) must read t before the PSUM recycles;
                # the qr16 ops are SBUF-only and deferred 2 steps so k8/v8
                # casts aren't queued behind them on the in-order DVE
                tmps = []
                for o in range(2):
                    tmp16 = small.tile([128, 512], F32, name="tmp16", tag="tmp16")
                    nc.vector.tensor_scalar(
                        tmp16, t[:, o, :], 16.0, bq16_sb[o], MULT, ADD
                    )
                    tmps.append(tmp16)
                pending_qr[s] = tmps

            def emit_q_resid(s):
                tmps = pending_qr.pop(s, None)
                if tmps is None:
                    return
                sl = slice(s * 512, (s + 1) * 512)
                for o in range(2):
                    # qr16 = tmp16 - 16*q8
                    nc.vector.affine_then_add(
                        qr16[:, o, sl], q8[:, o, sl], tmps[o], -16.0, 0.0
                    )

            def emit_v_pair(j):
                t = stp.tile([128, 2, 512], F32, name="vproj", tag="st")
                for h in range(2):
                    m = 2 * j + h
                    for i in range(2):
                        nc.tensor.matmul(
                            t[:, h, 0:C],
                            x_b[i][:, m * 128 : (m + 1) * 128],
                            w_b["v"][i],
                            start=(i == 0),
                            stop=(i == 1),
                        )
                # biasless v8 cast; alternate engines to balance DVE/Act load
                if j % 2 == 0:
                    nc.vector.tensor_copy(v8[:, 2 * j : 2 * j + 2, :], t[:, :, 0:C])
                else:
                    nc.scalar.activation(
                        v8[:, 2 * j : 2 * j + 2, :], t[:, :, 0:C], AF.Copy
                    )

            def emit_kt_pair(j):
                t = stp.tile([128, 2, 512], F32, name="ktproj", tag="st")
                for h in range(2):
                    m = 2 * j + h
                    for i in range(2):
                        nc.tensor.matmul(
                            t[:, h, 0:C],
                            x_b[i][:, m * 128 : (m + 1) * 128],
                            w_b["k"][i],
                            start=(i == 0),
                            stop=(i == 1),
                        )
                nc.vector.tensor_copy(kt8[:, 2 * j : 2 * j + 2, :], t[:, :, 0:C])

            # ---- G = (kt8^T v8)/16 accumulated in the spare psQ bank ----
            g_ps = {}

            def emit_g_matmul(half, j):
                if j == 0:
                    g_ps[half] = psQ.tile(
                        [128, C], F32, name=f"gps{half}", tag="rb"
                    )
                nc.tensor.matmul(
                    g_ps[half],
                    kt8[:, 2 * j : 2 * j + 2, half * 128 : (half + 1) * 128],
                    v8[:, 2 * j : 2 * j + 2, :],
                    start=(j == 0),
                    stop=(j == NJ - 1),
                    perf_mode=DR,
                )
                if j == NJ - 1:
                    nc.scalar.activation(
                        G8[:, half, :], g_ps[half], AF.Copy, scale=1.0 / 16.0
                    )

            # ---- main pipeline over (nt, j) steps ----
            pts = {}
            cur = {}
            pending_b = []  # (due_step, nt, u0, u1, rinv_r)
            last_corr = {}  # nt -> corr tile emitted early

            def emit_dn_corr(nt):
                # dn = qr16^T G8 slice; corr = dn/65536 + bv
                #     = (dq @ G)/4096 + bv   (bv rides through softmax intact)
                dn = stp.tile([128, 2, 512], F32, name="dn", tag="st")
                for ch in range(2):
                    nc.tensor.matmul(
                        dn[:, ch, :],
                        G8[:, :, ch * 128 : (ch + 1) * 128],
                        qr16[:, :, nt * 512 : (nt + 1) * 512],
                        start=True,
                        stop=True,
                        perf_mode=DR,
                    )
                # corr = dn/65536 + bv on the DVE (Act is exp-saturated in
                # steady state; bv rides through row-normalized attention)
                corr = small.tile([128, 2, 512], F32, name="corr", tag="corr")
                for ch in range(2):
                    nc.vector.tensor_scalar(
                        corr[:, ch, :],
                        dn[:, ch, :],
                        1.0 / 65536.0,
                        bv_col[ch],
                        MULT,
                        ADD,
                    )
                return corr

            def emit_ot(nt, j):
                pt_t = pts.pop((nt, j))
                if j == 0:
                    ot0 = psO.tile([128, 512], F32, name="ot0", tag="ot0")
                    ot1 = psO.tile([128, 512], F32, name="ot1", tag="ot1")
                    rs = psR.tile([32, 512], F32, name="rs", tag="rs")
                    cur[nt] = (ot0, ot1, rs)
                ot0, ot1, rs = cur[nt]
                st_, sp_ = (j == 0), (j == NJ - 1)
                nc.tensor.matmul(
                    ot0, v8[:, 2 * j : 2 * j + 2, 0:128], pt_t,
                    start=st_, stop=sp_, perf_mode=DR,
                )
                nc.tensor.matmul(
                    ot1, v8[:, 2 * j : 2 * j + 2, 128:256], pt_t,
                    start=st_, stop=sp_, perf_mode=DR,
                )
                nc.tensor.matmul(
                    rs, ones8, pt_t, start=st_, stop=sp_, perf_mode=DR,
                )

            def emit_epilogue_a(step, nt):
                ot0, ot1, rs = cur.pop(nt)
                # nt 0/1 wait for G8 (ready after step ~2*NJ+2); later tiles
                # use the normal short deferral
                due = step + EPI_DELAY
                if nt <= 2:
                    # G8 half-1 finishes accumulating at step 3*NJ-1 (tile 2);
                    # stagger the three backed-up epilogues so their PE/Act/DVE
                    # bursts don't pile onto consecutive steps
                    due = max(due, 3 * NJ + 2 + 4 * nt)
                if nt == NT - 1:
                    # final tile: dn/corr were emitted early; run the rinv ->
                    # rb -> mul/add -> DMA chain in 256-column chunks so DVE,
                    # PE and the output DMA pipeline instead of serializing.
                    corr = last_corr.pop(nt)
                    for hh in range(2):
                        cs = slice(hh * 256, (hh + 1) * 256)
                        rinv_f = small.tile(
                            [1, 256], F32, name="rinv_f", tag="rinv_f"
                        )
                        nc.vector.reciprocal_approx_fast(rinv_f, rs[0:1, cs])
                        rinv_r = small.tile(
                            [1, 256], FP16, name="rinv_r", tag="rinv_r"
                        )
                        nc.vector.tensor_copy(rinv_r, rinv_f)
                        rb = psQ.tile([128, 256], F32, name="rbh", tag="rb")
                        nc.tensor.matmul(
                            rb, ones_row, rinv_r, start=True, stop=True
                        )
                        rb_sb = small.tile(
                            [128, 256], F32, name="rb_sb", tag="rb_sb"
                        )
                        nc.vector.tensor_copy(rb_sb, rb)
                        for oc, u in ((0, ot0), (1, ot1)):
                            osb = osbp.tile(
                                [128, 256], F32, name="osb", tag=f"osb{oc}"
                            )
                            nc.vector.tensor_mul(osb, u[:, cs], rb_sb)
                            osb2 = osbp.tile(
                                [128, 256], F32, name="osb2", tag=f"osb2{oc}"
                            )
                            nc.vector.tensor_add(osb2, osb, corr[:, oc, cs])
                            nc.sync.dma_start(
                                out=out_d[
                                    oc * 128 : (oc + 1) * 128,
                                    nt * 512 + hh * 256 : nt * 512
                                    + (hh + 1) * 256,
                                ],
                                in_=osb2,
                            )
                    return
                u0 = up.tile([128, 512], F32, name="u0", tag="u0")
                nc.vector.tensor_copy(u0, ot0)
                u1 = up.tile([128, 512], F32, name="u1", tag="u1")
                nc.vector.tensor_copy(u1, ot1)
                rinv_f = small.tile([1, 512], F32, name="rinv_f", tag="rinv_f")
                nc.vector.reciprocal_approx_fast(rinv_f, rs[0:1, :])
                rinv_r = small.tile([1, 512], FP16, name="rinv_r", tag="rinv_r")
                nc.vector.tensor_copy(rinv_r, rinv_f)
                pending_b.append((due, nt, u0, u1, rinv_r))

            def emit_epilogue_b(nt, u0, u1, rinv_r):
                # (dn emitted before rb so the PE isn't gated on the rinv chain)
                corr = emit_dn_corr(nt)
                rb = psQ.tile([128, 512], F32, name="rb", tag="rb")
                nc.tensor.matmul(rb, ones_row, rinv_r, start=True, stop=True)
                for oc, u in ((0, u0), (1, u1)):
                    osb = osbp.tile([128, 512], F32, name="osb", tag=f"osb{oc}")
                    nc.vector.tensor_mul(osb, u, rb)
                    osb2 = osbp.tile([128, 512], F32, name="osb2", tag=f"osb2{oc}")
                    nc.vector.tensor_add(osb2, osb, corr[:, oc, :])
                    nc.sync.dma_start(
                        out=out_d[
                            oc * 128 : (oc + 1) * 128, nt * 512 : (nt + 1) * 512
                        ],
                        in_=osb2,
                    )

            k_emitted = 0
            q_emitted = 0
            TOTAL = NT * NJ
            for step in range(TOTAL + LAG):
                while pending_b and pending_b[0][0] <= step:
                    _, nt_, u0_, u1_, rv_ = pending_b.pop(0)
                    emit_epilogue_b(nt_, u0_, u1_, rv_)

                if step < TOTAL:
                    nt, j = divmod(step, NJ)
                    if nt == 0:
                        need_k = min(NT, (2 * j + 1) // 4 + 2)
                        while k_emitted < need_k:
                            emit_k_slice(k_emitted, halves=2 if k_emitted == 0 else 1)
                            k_emitted += 1
                        if q_emitted == 0:
                            emit_q_slice(0)
                            q_emitted = 1
                        if j == 1:
                            emit_ones8()
                        elif j == 2:
                            emit_ones_row()
                            emit_q_resid(0)
                    if j == 8 and nt < NT - 1:
                        emit_q_slice(nt + 1)
                        q_emitted += 1
                    elif j == 10 and nt < NT - 1:
                        emit_q_resid(nt + 1)
                    # scores (fp8 DoubleRow) + exp
                    st_t = stp.tile([128, 2, 512], F32, name="st_t", tag="st")
                    for h in range(2):
                        m = 2 * j + h
                        nc.tensor.matmul(
                            st_t[:, h, :],
                            k8[:, :, m * 128 : (m + 1) * 128],
                            q8[:, :, nt * 512 : (nt + 1) * 512],
                            start=True,
                            stop=True,
                            perf_mode=DR,
                        )
                    pt_t = ptp.tile([128, 2, 512], FP8, name="pt_t", tag="pt")
                    nc.scalar.activation(pt_t, st_t, AF.Exp, scale=SCALE)
                    pts[(nt, j)] = pt_t
                    if nt == 0:
                        emit_v_pair(j)
                    elif nt == 1:
                        # pixels-major K proj + G accumulation ride tile 1
                        emit_kt_pair(j)
                        emit_g_matmul(0, j)
                    elif nt == 2:
                        emit_g_matmul(1, j)
                    elif nt == NT - 1 and j == 2:
                        # final tile's correction, computed well before the
                        # tail so the last epilogue is just rinv/rb/mul/DMA
                        last_corr[nt] = emit_dn_corr(nt)

                if step >= LAG:
                    nt2, j2 = divmod(step - LAG, NJ)
                    emit_ot(nt2, j2)
                    if j2 == NJ - 1:
                        emit_epilogue_a(step, nt2)

            while pending_b:
                _, nt_, u0_, u1_, rv_ = pending_b.pop(0)
                emit_epilogue_b(nt_, u0_, u1_, rv_)

    nc.compile()
    return nc


def _get_nc():
    if "nc" not in _CACHE:
        _CACHE["nc"] = _build()
    return _CACHE["nc"]


def make_in_maps(x, wq, wk, wv, bq, bv):
    bf = ml_dtypes.bfloat16
    shared = {
        "wq_t": np.ascontiguousarray(wq.T.astype(bf)),
        "wk_t": np.ascontiguousarray(wk.T.astype(bf)),
        "wv_t": np.ascontiguousarray(wv.T.astype(bf)),
        "bq": np.ascontiguousarray(bq.reshape(C, 1).astype(np.float32)),
        "bv": np.ascontiguousarray(bv.reshape(1, C).astype(np.float32)),
    }
    return [
        {
            "x": np.ascontiguousarray(x[b].reshape(C, NPIX).astype(bf)),
            **shared,
        }
        for b in range(B)
    ]


def kernel(x, wq, wk, wv, bq, bk, bv):
    x = np.asarray(x, dtype=np.float32)
    wq = np.asarray(wq, dtype=np.float32)
    wk = np.asarray(wk, dtype=np.float32)
    wv = np.asarray(wv, dtype=np.float32)
    bq = np.asarray(bq, dtype=np.float32)
    bv = np.asarray(bv, dtype=np.float32)

    nc = _get_nc()
    in_maps = make_in_maps(x, wq, wk, wv, bq, bv)
    res = run_bass_kernel_spmd(nc, in_maps, core_ids=list(range(B)))
    out = np.stack([res.results[b]["out"] for b in range(B)])
    return out.reshape(B, C, 64, 64)
